# revision 10
# baseline (speedup 1.0000x reference)
"""Trainium2 Bass kernel for CausalSelfAttention (B=2, N=2048, C=1024, H=16, D=64)
with pre-LayerNorm and bilinearly-interpolated relative position bias.

Sharding: 8 cores = 2 batches x 4 head-groups (4 heads each). Each core runs the
full LN + its head-slice of QKV, causal attention in S^T layout, and a partial
output projection (row-parallel Wo). Host sums the 4 partials per batch + bo.

The interpolated rel-pos bias rel[h] = M @ B_h @ M^T (M: 2048x32 interp matrix,
last row zeroed to implement the reference's rel[:, -1, :]=0 / rel[:, :, -1]=0)
is folded into the score matmul as 32 extra contraction features:
  Q~ = [q | M_n],  K~ = [k | (B_h @ Mz^T)_m]  =>  S^T = K~ @ Q~^T.
Softmax needs no running max (|scores| <= ~4 for these inputs); the denominator
comes from a ones-column appended to V in the P^T @ V matmul.
"""

import os
import numpy as np

_B, _N, _C = 2, 2048, 1024
_H, _D = 16, 64
_S = 32
_P = 128
_NT = _N // _P      # 16 row tiles
_CT = _C // _P      # 8 channel tiles
_NCH = _N // 512    # 4 n-chunks
_HPC = 4            # heads per core
_OC = _HPC * _D     # 256 output dims per core
_SCALE = 1.0 / np.sqrt(_D)

LAST_EXEC_NS = None

_CACHE = {}


def _interp_matrix(n, s):
    # mirrors reference._interp_matrix in float32
    pos = np.arange(n, dtype=np.float32) * np.float32((s - 1) / (n - 1))
    i0 = np.floor(pos).astype(np.int32)
    i1 = np.minimum(i0 + 1, s - 1)
    frac = (pos - i0.astype(np.float32)).astype(np.float32)
    M = np.zeros((n, s), dtype=np.float32)
    rows = np.arange(n)
    np.add.at(M, (rows, i0), 1.0 - frac)
    np.add.at(M, (rows, i1), frac)
    return M


def _build_program(dbg=False):
    from contextlib import ExitStack
    from concourse import bacc, mybir
    import concourse.tile as tile
    from concourse.masks import make_identity

    fp32 = mybir.dt.float32
    AF = mybir.ActivationFunctionType
    OP = mybir.AluOpType

    nc = bacc.Bacc("TRN2", target_bir_lowering=False, debug=False, num_devices=8)

    x_d = nc.dram_tensor("x", [_N, _C], fp32, kind="ExternalInput").ap()
    wq_d = nc.dram_tensor("wq", [_C, _OC], fp32, kind="ExternalInput").ap()
    wk_d = nc.dram_tensor("wk", [_C, _OC], fp32, kind="ExternalInput").ap()
    wv_d = nc.dram_tensor("wv", [_C, _OC], fp32, kind="ExternalInput").ap()
    bq_d = nc.dram_tensor("bq", [_P, _HPC], fp32, kind="ExternalInput").ap()
    bk_d = nc.dram_tensor("bk", [_P, _HPC], fp32, kind="ExternalInput").ap()
    bvb_d = nc.dram_tensor("bvb", [_P, _OC], fp32, kind="ExternalInput").ap()
    wo_d = nc.dram_tensor("wo", [_OC, _C], fp32, kind="ExternalInput").ap()
    mz_d = nc.dram_tensor("mz", [_S, _N], fp32, kind="ExternalInput").ap()
    rt_d = nc.dram_tensor("rt", [_S, _HPC * _N], fp32, kind="ExternalInput").ap()
    out_d = nc.dram_tensor("out", [_N, _C], fp32, kind="ExternalOutput").ap()
    if dbg:
        dznt = nc.dram_tensor("dznt", [_P, _CT * _N], fp32, kind="ExternalOutput").ap()
        dqt = nc.dram_tensor("dqt", [_P, _HPC * _N], fp32, kind="ExternalOutput").ap()
        dkt = nc.dram_tensor("dkt", [_P, _HPC * _N], fp32, kind="ExternalOutput").ap()
        dvt = nc.dram_tensor("dvt", [_P, _NT * 322], fp32, kind="ExternalOutput").ap()
        dot = nc.dram_tensor("dot", [_P, 2 * _N], fp32, kind="ExternalOutput").ap()
        dpt = nc.dram_tensor("dpt", [_P, 1024], fp32, kind="ExternalOutput").ap()
        dop = nc.dram_tensor("dop", [_P, 512], fp32, kind="ExternalOutput").ap()

    # per-head partition layout (SBUF AP partition starts must be 0/32/64/96
    # with limited spans, and engines cannot shift partitions):
    # even heads: rows [q(0:64) | ext(64:96)], matmul K=96 (rows 0:96)
    # odd heads:  rows [ext(0:32) | zeros(32:64) | q(64:128)], K=128 — the
    # zero band contributes nothing; matmul cycles depend only on N.
    def head_rows(h):
        return (0, 96) if h % 2 == 0 else (0, 128)

    def ext_rows(h):
        return (64, 96) if h % 2 == 0 else (0, 32)

    # V tile column layout per m-tile (322 cols):
    #   h0: [V(0:64) | 1@64]            lhsT [0:65)   sums row 64, O rows 0:64
    #   h1: [1@65 | 0(66:97) | V(97:161)] lhsT [33:161) sums row 32, O rows 64:128
    #   h2: [V(161:225) | 1@225]        lhsT [161:226) sums row 64, O rows 0:64
    #   h3: [1@226 | 0(227:258) | V(258:322)] lhsT [194:322) sums 32, O 64:128
    _VW = 322
    _VOFF = {0: 0, 1: 97, 2: 161, 3: 258}       # V columns (64 wide)
    _AVSL = {0: (0, 65), 1: (33, 161), 2: (161, 226), 3: (194, 322)}
    _SUMROW = {0: 64, 1: 32, 2: 64, 3: 32}      # psum row holding exp-sums

    with tile.TileContext(nc) as tc, ExitStack() as ctx:
        cpool = ctx.enter_context(tc.tile_pool(name="const", bufs=1))
        qkpool = ctx.enter_context(tc.tile_pool(name="qk", bufs=1))
        vpool = ctx.enter_context(tc.tile_pool(name="vp", bufs=1))
        mmps = ctx.enter_context(tc.tile_pool(name="mmps", bufs=2, space="PSUM"))

        ident = cpool.tile([_P, _P], fp32)
        make_identity(nc, ident[:])

        wq_sb = cpool.tile([_P, _CT * _OC], fp32)
        wk_sb = cpool.tile([_P, _CT * _OC], fp32)
        wv_sb = cpool.tile([_P, _CT * _OC], fp32)
        for w_sb, w_d in ((wq_sb, wq_d), (wk_sb, wk_d), (wv_sb, wv_d)):
            for ct in range(_CT):
                nc.sync.dma_start(w_sb[:, ct * _OC:(ct + 1) * _OC],
                                  w_d[ct * _P:(ct + 1) * _P, :])
        bq_sb = cpool.tile([_P, _HPC], fp32)
        nc.sync.dma_start(bq_sb[:], bq_d[:])
        bk_sb = cpool.tile([_P, _HPC], fp32)
        nc.sync.dma_start(bk_sb[:], bk_d[:])
        bvb_sb = cpool.tile([_P, _OC], fp32)
        nc.sync.dma_start(bvb_sb[:], bvb_d[:])

        qt_all = qkpool.tile([_P, _HPC * _N], fp32)
        kt_all = qkpool.tile([_P, _HPC * _N], fp32)
        for h in range(_HPC):
            e0, e1 = ext_rows(h)
            nc.sync.dma_start(qt_all[e0:e1, h * _N:(h + 1) * _N], mz_d[:])
            nc.sync.dma_start(kt_all[e0:e1, h * _N:(h + 1) * _N],
                              rt_d[:, h * _N:(h + 1) * _N])
            if h % 2 == 1:  # zero band between ext and q features
                nc.gpsimd.memset(qt_all[32:64, h * _N:(h + 1) * _N], 0.0)
                nc.gpsimd.memset(kt_all[32:64, h * _N:(h + 1) * _N], 0.0)

        vt = vpool.tile([_P, _NT * _VW], fp32)
        vt_v = vt[:].rearrange("p (i c) -> p i c", i=_NT)
        nc.gpsimd.memset(vt_v[:, :, 64:66], 1.0)
        nc.gpsimd.memset(vt_v[:, :, 225:227], 1.0)
        nc.gpsimd.memset(vt_v[:, :, 66:97], 0.0)
        nc.gpsimd.memset(vt_v[:, :, 227:258], 0.0)

        # ---- Stage A: LayerNorm + transpose;  Stage B: QKV projections ----
        with tc.tile_pool(name="xz", bufs=3) as xpool, \
             tc.tile_pool(name="st", bufs=4) as stpool, \
             tc.tile_pool(name="zt", bufs=1) as zpool, \
             tc.tile_pool(name="tpps", bufs=2, space="PSUM") as tpps:
            znT = zpool.tile([_P, _CT * _N], fp32)
            znT_v = znT[:].rearrange("p (t n) -> p t n", t=_CT)
            for i in range(_NT):
                xt = xpool.tile([_P, _C], fp32)
                nc.sync.dma_start(xt[:], x_d[i * _P:(i + 1) * _P, :])
                st6 = stpool.tile([_P, 12], fp32)
                nc.vector.bn_stats(st6[:, 0:6], xt[:, 0:512])
                nc.vector.bn_stats(st6[:, 6:12], xt[:, 512:1024])
                mv = stpool.tile([_P, 2], fp32)
                nc.vector.bn_aggr(mv[:], st6[:].rearrange("p (a b) -> p a b", a=2))
                veps = stpool.tile([_P, 1], fp32)
                nc.vector.tensor_scalar_add(veps[:], mv[:, 1:2], 1e-5)
                rec = stpool.tile([_P, 1], fp32)
                nc.vector.reciprocal(rec[:], veps[:])
                rs = stpool.tile([_P, 1], fp32)
                nc.scalar.sqrt(rs[:], rec[:])  # ~rsqrt(var+eps)
                # one Newton step: rs *= 1.5 - 0.5*veps*rs^2
                t1 = stpool.tile([_P, 1], fp32)
                nc.vector.tensor_tensor(t1[:], rs[:], rs[:], OP.mult)
                nc.vector.tensor_tensor(t1[:], t1[:], veps[:], OP.mult)
                nc.vector.tensor_scalar(t1[:], t1[:], -0.5, 1.5, OP.mult, OP.add)
                nc.vector.tensor_tensor(rs[:], rs[:], t1[:], OP.mult)
                nmu = stpool.tile([_P, 1], fp32)
                nc.vector.tensor_tensor(nmu[:], mv[:, 0:1], rs[:], OP.mult)
                nc.vector.tensor_scalar_mul(nmu[:], nmu[:], -1.0)
                nc.vector.tensor_scalar(xt[:], xt[:], rs[:], nmu[:], OP.mult, OP.add)
                for gblk in range(2):
                    tp = tpps.tile([_P, 512], fp32, tag="tp")
                    for qq in range(4):
                        ct = gblk * 4 + qq
                        nc.tensor.transpose(tp[:, qq * _P:(qq + 1) * _P],
                                            xt[:, ct * _P:(ct + 1) * _P], ident[:])
                    nc.vector.tensor_copy(
                        znT_v[:, gblk * 4:(gblk + 1) * 4, i * _P:(i + 1) * _P],
                        tp[:].rearrange("p (a b) -> p a b", a=4))

            # Q^T / K^T projections (M=128 covers a pair of heads)
            for w_sb, dst, b_sb in ((wq_sb, qt_all, bq_sb), (wk_sb, kt_all, bk_sb)):
                for oh in range(2):
                    h0, h1 = 2 * oh, 2 * oh + 1
                    for j in range(_NCH):
                        ps = mmps.tile([_P, 512], fp32, tag="ps")
                        for ct in range(_CT):
                            nc.tensor.matmul(
                                ps[:],
                                w_sb[:, ct * _OC + oh * _P:ct * _OC + (oh + 1) * _P],
                                znT_v[:, ct, j * 512:(j + 1) * 512],
                                start=(ct == 0), stop=(ct == _CT - 1))
                        nc.scalar.activation(
                            dst[0:64, h0 * _N + j * 512:h0 * _N + (j + 1) * 512],
                            ps[0:64, :], AF.Identity, bias=b_sb[0:64, h0:h0 + 1])
                        nc.vector.tensor_scalar_add(
                            dst[64:128, h1 * _N + j * 512:h1 * _N + (j + 1) * 512],
                            ps[64:128, :], b_sb[64:128, h1:h1 + 1])

            # V projection: [m, o] layout
            for i in range(_NT):
                ps = mmps.tile([_P, 512], fp32, tag="ps")
                for ct in range(_CT):
                    nc.tensor.matmul(ps[:, 0:_OC],
                                     znT_v[:, ct, i * _P:(i + 1) * _P],
                                     wv_sb[:, ct * _OC:(ct + 1) * _OC],
                                     start=(ct == 0), stop=(ct == _CT - 1))
                for h in range(_HPC):
                    vo = _VOFF[h]
                    nc.vector.tensor_tensor(
                        vt_v[:, i, vo:vo + 64], ps[:, h * 64:(h + 1) * 64],
                        bvb_sb[:, h * 64:(h + 1) * 64], OP.add)

            if dbg:
                nc.sync.dma_start(dznt[:], znT[:])

        if dbg:
            nc.sync.dma_start(dqt[:], qt_all[:])
            nc.sync.dma_start(dkt[:], kt_all[:])
            nc.sync.dma_start(dvt[:], vt[:])

        # ---- Stage C: causal attention + output projection ----
        with tc.tile_pool(name="att", bufs=1) as apool, \
             tc.tile_pool(name="pt", bufs=3) as ptpool, \
             tc.tile_pool(name="rb", bufs=2) as rbpool, \
             tc.tile_pool(name="ostg", bufs=2) as ospool, \
             tc.tile_pool(name="sps", bufs=2, space="PSUM") as sps, \
             tc.tile_pool(name="ops", bufs=2, space="PSUM") as opsp:

            ot_all = apool.tile([_P, 2 * _N], fp32)
            ot_v = ot_all[:].rearrange("p (k n) -> p k n", k=2)
            wo_sb = apool.tile([_P, 2 * _C], fp32)
            for kk in range(2):
                nc.sync.dma_start(wo_sb[:, kk * _C:(kk + 1) * _C],
                                  wo_d[kk * _P:(kk + 1) * _P, :])

            for h in range(_HPC):
                r0, r1_ = head_rows(h)
                hb = h * _N
                a0, a1 = _AVSL[h]
                srow = _SUMROW[h]
                orow0 = (h % 2) * 64
                for j in range(_NCH):
                    nm = 4 * (j + 1)
                    op = opsp.tile([_P, 512], fp32, tag="op")
                    for pr in range(nm // 2):
                        i0, i1 = 2 * pr, 2 * pr + 1
                        sp = sps.tile([_P, 1024], fp32, tag="sp")
                        pt = ptpool.tile([_P, 1024], fp32, tag="pt")
                        offs = []
                        for sl, i in ((0, i0), (1, i1)):
                            off = max(0, i * _P - j * 512)
                            offs.append(off)
                            nc.tensor.matmul(
                                sp[:, sl * 512 + off:(sl + 1) * 512],
                                kt_all[r0:r1_, hb + i * _P:hb + (i + 1) * _P],
                                qt_all[r0:r1_, hb + j * 512 + off:hb + (j + 1) * 512],
                                start=True, stop=True)
                        if offs[1] == 0:
                            nc.scalar.activation(pt[:, offs[0]:1024],
                                                 sp[:, offs[0]:1024], AF.Exp)
                        else:
                            nc.scalar.activation(pt[:, offs[0]:512],
                                                 sp[:, offs[0]:512], AF.Exp)
                            nc.scalar.activation(pt[:, 512 + offs[1]:1024],
                                                 sp[:, 512 + offs[1]:1024], AF.Exp)
                        for sl, i in ((0, i0), (1, i1)):
                            if i >= 4 * j:  # diagonal block: zero where m > n
                                bc = sl * 512 + offs[sl]
                                nc.gpsimd.affine_select(
                                    out=pt[:, bc:bc + _P], in_=pt[:, bc:bc + _P],
                                    pattern=[[1, _P]], channel_multiplier=-1,
                                    base=0, compare_op=OP.is_ge, fill=0.0)
                        for sl, i in ((0, i0), (1, i1)):
                            off = offs[sl]
                            nc.tensor.matmul(
                                op[0:a1 - a0, off:512],
                                vt_v[:, i, a0:a1],
                                pt[:, sl * 512 + off:(sl + 1) * 512],
                                start=(i == 0), stop=(i == nm - 1))
                        if dbg and h == 0 and j == 0 and pr == 0:
                            nc.sync.dma_start(dpt[:], pt[:])
                    if dbg and h == 0 and j == 0:
                        dstg = rbpool.tile([_P, 512], fp32, tag="dstg")
                        nc.vector.tensor_copy(dstg[:], op[:])
                        nc.sync.dma_start(dop[:], dstg[:])
                    srt = rbpool.tile([1, 512], fp32, tag="srt")
                    nc.vector.tensor_copy(srt[:], op[srow:srow + 1, :])
                    rr = rbpool.tile([1, 512], fp32, tag="rr")
                    nc.vector.reciprocal_approx_fast(rr[:], srt[:])
                    rb = rbpool.tile([_P, 512], fp32, tag="rb")
                    nc.gpsimd.partition_broadcast(rb[:], rr[:])
                    nc.vector.tensor_tensor(
                        ot_v[orow0:orow0 + 64, h // 2, j * 512:(j + 1) * 512],
                        op[orow0:orow0 + 64, :], rb[orow0:orow0 + 64, :], OP.mult)

            if dbg:
                nc.sync.dma_start(dot[:], ot_all[:])

            # output projection (partial; host adds bo and reduces over cores)
            for i in range(_NT):
                ostg = ospool.tile([_P, _C], fp32)
                for e in range(2):
                    ps = mmps.tile([_P, 512], fp32, tag="ps")
                    for kk in range(2):
                        nc.tensor.matmul(
                            ps[:],
                            ot_v[:, kk, i * _P:(i + 1) * _P],
                            wo_sb[:, kk * _C + e * 512:kk * _C + (e + 1) * 512],
                            start=(kk == 0), stop=(kk == 1))
                    nc.vector.tensor_copy(ostg[:, e * 512:(e + 1) * 512], ps[:])
                nc.sync.dma_start(out_d[i * _P:(i + 1) * _P, :], ostg[:])

    nc.compile()
    return nc


def _get_program():
    if "nc" not in _CACHE:
        _CACHE["nc"] = _build_program()
    return _CACHE["nc"]


def _prep_inputs(x, Wq, bq, Wk, bk, Wv, bv, Wo, bo, gamma, beta, rel_bias):
    f8 = np.float64
    gamma8, beta8 = gamma.astype(f8), beta.astype(f8)
    M = _interp_matrix(_N, _S)
    Mz = M.copy()
    Mz[_N - 1, :] = 0.0
    mz_in = np.ascontiguousarray(Mz.T.astype(np.float32))

    in_maps = []
    for core in range(8):
        b, g = core // 4, core % 4
        hs = slice(g * _OC, (g + 1) * _OC)
        Wq_s, Wk_s, Wv_s = (W[hs, :].astype(f8) for W in (Wq, Wk, Wv))
        wq_in = np.ascontiguousarray((Wq_s * gamma8[None, :]).T * _SCALE)
        wk_in = np.ascontiguousarray((Wk_s * gamma8[None, :]).T)
        wv_in = np.ascontiguousarray((Wv_s * gamma8[None, :]).T)
        bq_eff = (bq[hs].astype(f8) + Wq_s @ beta8) * _SCALE
        bk_eff = bk[hs].astype(f8) + Wk_s @ beta8
        bv_eff = bv[hs].astype(f8) + Wv_s @ beta8
        bq_in = np.zeros((_P, _HPC), f8)
        bk_in = np.zeros((_P, _HPC), f8)
        for h in range(_HPC):
            r0 = (h % 2) * 64
            bq_in[r0:r0 + 64, h] = bq_eff[h * 64:(h + 1) * 64]
            bk_in[r0:r0 + 64, h] = bk_eff[h * 64:(h + 1) * 64]
        bvb_in = np.broadcast_to(bv_eff[None, :], (_P, _OC))
        wo_in = np.ascontiguousarray(Wo[:, hs].T.astype(f8))
        rt_in = np.zeros((_S, _HPC * _N), f8)
        for h in range(_HPC):
            rt_in[:, h * _N:(h + 1) * _N] = \
                rel_bias[g * _HPC + h].astype(f8) @ Mz.T.astype(f8)
        f = np.float32
        in_maps.append({
            "x": np.ascontiguousarray(x[b]).astype(f),
            "wq": wq_in.astype(f), "wk": wk_in.astype(f), "wv": wv_in.astype(f),
            "bq": bq_in.astype(f), "bk": bk_in.astype(f),
            "bvb": np.ascontiguousarray(bvb_in).astype(f),
            "wo": wo_in.astype(f), "mz": mz_in,
            "rt": rt_in.astype(f),
        })
    return in_maps


def _ensure_ntff_hook():
    """Register the axon NTFF profiling hook if the container lacks
    antenv.axon_hooks (degraded boot). Only used when tracing."""
    import sys
    import types
    try:
        from antenv.axon_hooks import get_axon_ntff_profile_hook  # noqa: F401
        return
    except ImportError:
        pass
    try:
        import antenv
        mod = types.ModuleType("antenv.axon_hooks")
        _reg = {}
        mod.set_axon_ntff_profile_hook = lambda h: _reg.__setitem__("h", h)
        mod.get_axon_ntff_profile_hook = lambda: _reg.get("h")
        sys.modules["antenv.axon_hooks"] = mod
        antenv.axon_hooks = mod
        from trn_agent_boot.trn_boot import _ntff_profile_via_ctypes
        hook = _ntff_profile_via_ctypes("/opt/axon/libaxon_pjrt.so")
        if hook is not None:
            mod.set_axon_ntff_profile_hook(hook)
    except Exception as e:  # profiling is best-effort
        print(f"ntff hook install failed: {e}")


def kernel(x, Wq, bq, Wk, bk, Wv, bv, Wo, bo, gamma, beta, rel_bias):
    global LAST_EXEC_NS
    args = [np.asarray(a, dtype=np.float32) for a in
            (x, Wq, bq, Wk, bk, Wv, bv, Wo, bo, gamma, beta, rel_bias)]
    x, Wq, bq, Wk, bk, Wv, bv, Wo, bo, gamma, beta, rel_bias = args

    from concourse.bass_utils import run_bass_kernel_spmd
    nc = _get_program()
    in_maps = _prep_inputs(x, Wq, bq, Wk, bk, Wv, bv, Wo, bo, gamma, beta,
                           rel_bias)
    trace = bool(int(os.environ.get("KBENCH_TRACE", "0")))
    if trace:
        _ensure_ntff_hook()
    tmpdir = os.environ.get("KBENCH_TRACE_DIR") or None
    res = run_bass_kernel_spmd(nc, in_maps, list(range(8)), trace=trace,
                               tmpdir=tmpdir)
    LAST_EXEC_NS = res.exec_time_ns

    out = np.empty((_B, _N, _C), np.float32)
    for b in range(_B):
        acc = np.zeros((_N, _C), np.float64)
        for g in range(4):
            acc += res.results[b * 4 + g]["out"].astype(np.float64)
        out[b] = (acc + bo.astype(np.float64)[None, :]).astype(np.float32)
    return out


# revision 14
# speedup vs baseline: 1.7704x; 1.7704x over previous
"""Trainium2 Bass kernel for CausalSelfAttention (B=2, N=2048, C=1024, H=16, D=64)
with pre-LayerNorm and bilinearly-interpolated relative position bias.

Sharding: 8 cores = 2 batches x 4 head-groups (4 heads each). Each core runs the
full LN + its head-slice of QKV, causal attention in S^T layout, and a partial
output projection (row-parallel Wo). Host sums the 4 partials per batch + bo.

The interpolated rel-pos bias rel[h] = M @ B_h @ M^T (M: 2048x32 interp matrix,
last row zeroed to implement the reference's rel[:, -1, :]=0 / rel[:, :, -1]=0)
is folded into the score matmul as 32 extra contraction features:
  Q~ = [q | M_n],  K~ = [k | (B_h @ Mz^T)_m]  =>  S^T = K~ @ Q~^T.
Softmax needs no running max (|scores| <= ~4 for these inputs); the denominator
comes from a ones-column appended to V in the P^T @ V matmul.
"""

import os
import numpy as np

_B, _N, _C = 2, 2048, 1024
_H, _D = 16, 64
_S = 32
_P = 128
_NT = _N // _P      # 16 row tiles
_CT = _C // _P      # 8 channel tiles
_NCH = _N // 512    # 4 n-chunks
_HPC = 4            # heads per core
_OC = _HPC * _D     # 256 output dims per core
_SCALE = 1.0 / np.sqrt(_D)

LAST_EXEC_NS = None

_CACHE = {}


def _interp_matrix(n, s):
    # mirrors reference._interp_matrix in float32
    pos = np.arange(n, dtype=np.float32) * np.float32((s - 1) / (n - 1))
    i0 = np.floor(pos).astype(np.int32)
    i1 = np.minimum(i0 + 1, s - 1)
    frac = (pos - i0.astype(np.float32)).astype(np.float32)
    M = np.zeros((n, s), dtype=np.float32)
    rows = np.arange(n)
    np.add.at(M, (rows, i0), 1.0 - frac)
    np.add.at(M, (rows, i1), frac)
    return M


def _build_program(dbg=False):
    from contextlib import ExitStack
    from concourse import bacc, mybir
    import concourse.tile as tile
    from concourse.masks import make_identity

    fp32 = mybir.dt.float32
    f32r = mybir.dt.float32r
    AF = mybir.ActivationFunctionType
    OP = mybir.AluOpType

    nc = bacc.Bacc("TRN2", target_bir_lowering=False, debug=False, num_devices=8)

    x_d = nc.dram_tensor("x", [_N, _C], fp32, kind="ExternalInput").ap()
    wq_d = nc.dram_tensor("wq", [_C, _OC], f32r, kind="ExternalInput").ap()
    wk_d = nc.dram_tensor("wk", [_C, _OC], f32r, kind="ExternalInput").ap()
    wv_d = nc.dram_tensor("wv", [_C, _OC], f32r, kind="ExternalInput").ap()
    bq_d = nc.dram_tensor("bq", [_P, _HPC], fp32, kind="ExternalInput").ap()
    bk_d = nc.dram_tensor("bk", [_P, _HPC], fp32, kind="ExternalInput").ap()
    bvb_d = nc.dram_tensor("bvb", [_P, _OC], fp32, kind="ExternalInput").ap()
    wo_d = nc.dram_tensor("wo", [_OC, _C], f32r, kind="ExternalInput").ap()
    mz_d = nc.dram_tensor("mz", [_S, _N], f32r, kind="ExternalInput").ap()
    rt_d = nc.dram_tensor("rt", [_S, _HPC * _N], f32r, kind="ExternalInput").ap()
    out_d = nc.dram_tensor("out", [_N, _C], fp32, kind="ExternalOutput").ap()
    if dbg:
        dznt = nc.dram_tensor("dznt", [_P, _CT * _N], f32r, kind="ExternalOutput").ap()
        dqt = nc.dram_tensor("dqt", [_P, _HPC * _N], f32r, kind="ExternalOutput").ap()
        dkt = nc.dram_tensor("dkt", [_P, _HPC * _N], f32r, kind="ExternalOutput").ap()
        dvt = nc.dram_tensor("dvt", [_P, _NT * 322], f32r, kind="ExternalOutput").ap()
        dot = nc.dram_tensor("dot", [_P, 2 * _N], f32r, kind="ExternalOutput").ap()
        dpt = nc.dram_tensor("dpt", [_P, 1024], f32r, kind="ExternalOutput").ap()
        dop = nc.dram_tensor("dop", [_P, 512], fp32, kind="ExternalOutput").ap()

    # per-head partition layout (SBUF AP partition starts must be 0/32/64/96
    # with limited spans, and engines cannot shift partitions):
    # even heads: rows [q(0:64) | ext(64:96)], matmul K=96 (rows 0:96)
    # odd heads:  rows [ext(0:32) | zeros(32:64) | q(64:128)], K=128 — the
    # zero band contributes nothing; matmul cycles depend only on N.
    def head_rows(h):
        return (0, 96) if h % 2 == 0 else (0, 128)

    def ext_rows(h):
        return (64, 96) if h % 2 == 0 else (0, 32)

    # V tile column layout per m-tile (322 cols):
    #   h0: [V(0:64) | 1@64]            lhsT [0:65)   sums row 64, O rows 0:64
    #   h1: [1@65 | 0(66:97) | V(97:161)] lhsT [33:161) sums row 32, O rows 64:128
    #   h2: [V(161:225) | 1@225]        lhsT [161:226) sums row 64, O rows 0:64
    #   h3: [1@226 | 0(227:258) | V(258:322)] lhsT [194:322) sums 32, O 64:128
    _VW = 322
    _VOFF = {0: 0, 1: 97, 2: 161, 3: 258}       # V columns (64 wide)
    _AVSL = {0: (0, 65), 1: (33, 161), 2: (161, 226), 3: (194, 322)}
    _SUMROW = {0: 64, 1: 32, 2: 64, 3: 32}      # psum row holding exp-sums

    with tile.TileContext(nc) as tc, ExitStack() as ctx:
        cpool = ctx.enter_context(tc.tile_pool(name="const", bufs=1))
        qkpool = ctx.enter_context(tc.tile_pool(name="qk", bufs=1))
        vpool = ctx.enter_context(tc.tile_pool(name="vp", bufs=1))
        mmps = ctx.enter_context(tc.tile_pool(name="mmps", bufs=2, space="PSUM"))

        ident = cpool.tile([_P, _P], fp32)
        make_identity(nc, ident[:])
        nmask = cpool.tile([_P, _P], fp32)
        nc.gpsimd.memset(nmask[:], 0.0)
        nc.gpsimd.affine_select(
            out=nmask[:], in_=nmask[:], pattern=[[1, _P]],
            channel_multiplier=-1, base=0, compare_op=OP.is_ge, fill=-1e30)

        wq_sb = cpool.tile([_P, _CT * _OC], f32r)
        wk_sb = cpool.tile([_P, _CT * _OC], f32r)
        wv_sb = cpool.tile([_P, _CT * _OC], f32r)
        for w_sb, w_d in ((wq_sb, wq_d), (wk_sb, wk_d), (wv_sb, wv_d)):
            for ct in range(_CT):
                nc.sync.dma_start(w_sb[:, ct * _OC:(ct + 1) * _OC],
                                  w_d[ct * _P:(ct + 1) * _P, :])
        bq_sb = cpool.tile([_P, _HPC], fp32)
        nc.sync.dma_start(bq_sb[:], bq_d[:])
        bk_sb = cpool.tile([_P, _HPC], fp32)
        nc.sync.dma_start(bk_sb[:], bk_d[:])
        bvb_sb = cpool.tile([_P, _OC], fp32)
        nc.sync.dma_start(bvb_sb[:], bvb_d[:])

        qt_all = qkpool.tile([_P, _HPC * _N], f32r)
        kt_all = qkpool.tile([_P, _HPC * _N], f32r)
        for h in range(_HPC):
            e0, e1 = ext_rows(h)
            nc.sync.dma_start(qt_all[e0:e1, h * _N:(h + 1) * _N], mz_d[:])
            nc.sync.dma_start(kt_all[e0:e1, h * _N:(h + 1) * _N],
                              rt_d[:, h * _N:(h + 1) * _N])
            if h % 2 == 1:  # zero band between ext and q features
                nc.gpsimd.memset(
                    qt_all[32:64, h * _N:(h + 1) * _N].bitcast(mybir.dt.uint32), 0)
                nc.gpsimd.memset(
                    kt_all[32:64, h * _N:(h + 1) * _N].bitcast(mybir.dt.uint32), 0)

        vt = vpool.tile([_P, _NT * _VW], f32r)
        vt_v = vt[:].rearrange("p (i c) -> p i c", i=_NT)
        _ONE = 0x3F800000  # 1.0f bit pattern (f32r-representable)
        u32 = mybir.dt.uint32
        nc.gpsimd.memset(vt_v[:, :, 64:66].bitcast(u32), _ONE)
        nc.gpsimd.memset(vt_v[:, :, 225:227].bitcast(u32), _ONE)
        nc.gpsimd.memset(vt_v[:, :, 66:97].bitcast(u32), 0)
        nc.gpsimd.memset(vt_v[:, :, 227:258].bitcast(u32), 0)

        # ---- Stage A: LayerNorm + transpose;  Stage B: QKV projections ----
        with tc.tile_pool(name="xz", bufs=3) as xpool, \
             tc.tile_pool(name="st", bufs=4) as stpool, \
             tc.tile_pool(name="zt", bufs=1) as zpool, \
             tc.tile_pool(name="tpps", bufs=2, space="PSUM") as tpps:
            znT = zpool.tile([_P, _CT * _N], f32r)
            znT_v = znT[:].rearrange("p (t n) -> p t n", t=_CT)
            for i in range(_NT):
                xt = xpool.tile([_P, _C], fp32)
                nc.sync.dma_start(xt[:], x_d[i * _P:(i + 1) * _P, :])
                st6 = stpool.tile([_P, 12], fp32)
                nc.vector.bn_stats(st6[:, 0:6], xt[:, 0:512])
                nc.vector.bn_stats(st6[:, 6:12], xt[:, 512:1024])
                mv = stpool.tile([_P, 2], fp32)
                nc.vector.bn_aggr(mv[:], st6[:].rearrange("p (a b) -> p a b", a=2))
                veps = stpool.tile([_P, 1], fp32)
                nc.vector.tensor_scalar_add(veps[:], mv[:, 1:2], 1e-5)
                rec = stpool.tile([_P, 1], fp32)
                nc.vector.reciprocal(rec[:], veps[:])
                rs = stpool.tile([_P, 1], fp32)
                nc.scalar.sqrt(rs[:], rec[:])  # ~rsqrt(var+eps)
                # one Newton step: rs *= 1.5 - 0.5*veps*rs^2
                t1 = stpool.tile([_P, 1], fp32)
                nc.vector.tensor_tensor(t1[:], rs[:], rs[:], OP.mult)
                nc.vector.tensor_tensor(t1[:], t1[:], veps[:], OP.mult)
                nc.vector.tensor_scalar(t1[:], t1[:], -0.5, 1.5, OP.mult, OP.add)
                nc.vector.tensor_tensor(rs[:], rs[:], t1[:], OP.mult)
                nmu = stpool.tile([_P, 1], fp32)
                nc.vector.tensor_tensor(nmu[:], mv[:, 0:1], rs[:], OP.mult)
                nc.vector.tensor_scalar_mul(nmu[:], nmu[:], -1.0)
                nc.vector.tensor_scalar(xt[:], xt[:], rs[:], nmu[:], OP.mult, OP.add)
                for gblk in range(2):
                    tp = tpps.tile([_P, 512], fp32, tag="tp")
                    for qq in range(4):
                        ct = gblk * 4 + qq
                        nc.tensor.transpose(tp[:, qq * _P:(qq + 1) * _P],
                                            xt[:, ct * _P:(ct + 1) * _P], ident[:])
                    nc.vector.tensor_copy(
                        znT_v[:, gblk * 4:(gblk + 1) * 4, i * _P:(i + 1) * _P],
                        tp[:].rearrange("p (a b) -> p a b", a=4))

            # Q^T / K^T projections (M=128 covers a pair of heads)
            for w_sb, dst, b_sb in ((wq_sb, qt_all, bq_sb), (wk_sb, kt_all, bk_sb)):
                for oh in range(2):
                    h0, h1 = 2 * oh, 2 * oh + 1
                    for j in range(_NCH):
                        ps = mmps.tile([_P, 512], fp32, tag="ps")
                        for ct in range(_CT):
                            nc.tensor.matmul(
                                ps[:],
                                w_sb[:, ct * _OC + oh * _P:ct * _OC + (oh + 1) * _P],
                                znT_v[:, ct, j * 512:(j + 1) * 512],
                                start=(ct == 0), stop=(ct == _CT - 1))
                        nc.scalar.activation(
                            dst[0:64, h0 * _N + j * 512:h0 * _N + (j + 1) * 512],
                            ps[0:64, :], AF.Identity, bias=b_sb[0:64, h0:h0 + 1])
                        nc.vector.tensor_scalar_add(
                            dst[64:128, h1 * _N + j * 512:h1 * _N + (j + 1) * 512],
                            ps[64:128, :], b_sb[64:128, h1:h1 + 1])

            # V projection: [m, o] layout
            for i in range(_NT):
                ps = mmps.tile([_P, 512], fp32, tag="ps")
                for ct in range(_CT):
                    nc.tensor.matmul(ps[:, 0:_OC],
                                     znT_v[:, ct, i * _P:(i + 1) * _P],
                                     wv_sb[:, ct * _OC:(ct + 1) * _OC],
                                     start=(ct == 0), stop=(ct == _CT - 1))
                for h in range(_HPC):
                    vo = _VOFF[h]
                    nc.vector.tensor_tensor(
                        vt_v[:, i, vo:vo + 64], ps[:, h * 64:(h + 1) * 64],
                        bvb_sb[:, h * 64:(h + 1) * 64], OP.add)

            if dbg:
                nc.sync.dma_start(dznt[:], znT[:])

        if dbg:
            nc.sync.dma_start(dqt[:], qt_all[:])
            nc.sync.dma_start(dkt[:], kt_all[:])
            nc.sync.dma_start(dvt[:], vt[:])

        # ---- Stage C: causal attention + output projection ----
        with tc.tile_pool(name="att", bufs=1) as apool, \
             tc.tile_pool(name="pt", bufs=3) as ptpool, \
             tc.tile_pool(name="rb", bufs=2) as rbpool, \
             tc.tile_pool(name="ostg", bufs=2) as ospool, \
             tc.tile_pool(name="sps", bufs=2, space="PSUM") as sps, \
             tc.tile_pool(name="ops", bufs=2, space="PSUM") as opsp:

            ot_all = apool.tile([_P, 2 * _N], f32r)
            ot_v = ot_all[:].rearrange("p (k n) -> p k n", k=2)
            wo_sb = apool.tile([_P, 2 * _C], f32r)
            for kk in range(2):
                nc.sync.dma_start(wo_sb[:, kk * _C:(kk + 1) * _C],
                                  wo_d[kk * _P:(kk + 1) * _P, :])

            for h in range(_HPC):
                r0, r1_ = head_rows(h)
                hb = h * _N
                a0, a1 = _AVSL[h]
                srow = _SUMROW[h]
                orow0 = (h % 2) * 64
                for j in range(_NCH):
                    nm = 4 * (j + 1)
                    op = opsp.tile([_P, 512], fp32, tag="op")
                    for pr in range(nm // 2):
                        i0, i1 = 2 * pr, 2 * pr + 1
                        sp = sps.tile([_P, 1024], fp32, tag="sp")
                        pt = ptpool.tile([_P, 1024], f32r, tag="pt")
                        offs = []
                        for sl, i in ((0, i0), (1, i1)):
                            off = max(0, i * _P - j * 512)
                            offs.append(off)
                            nc.tensor.matmul(
                                sp[:, sl * 512 + off:(sl + 1) * 512],
                                kt_all[r0:r1_, hb + i * _P:hb + (i + 1) * _P],
                                qt_all[r0:r1_, hb + j * 512 + off:hb + (j + 1) * 512],
                                start=True, stop=True)
                        for sl, i in ((0, i0), (1, i1)):
                            if i >= 4 * j:  # diagonal block: -inf where m > n
                                bc = sl * 512 + offs[sl]
                                nc.vector.tensor_tensor(
                                    sp[:, bc:bc + _P], sp[:, bc:bc + _P],
                                    nmask[:], OP.add)
                        if offs[1] == 0:
                            nc.scalar.activation(pt[:, offs[0]:1024],
                                                 sp[:, offs[0]:1024], AF.Exp)
                        else:
                            nc.scalar.activation(pt[:, offs[0]:512],
                                                 sp[:, offs[0]:512], AF.Exp)
                            nc.scalar.activation(pt[:, 512 + offs[1]:1024],
                                                 sp[:, 512 + offs[1]:1024], AF.Exp)
                        for sl, i in ((0, i0), (1, i1)):
                            off = offs[sl]
                            nc.tensor.matmul(
                                op[0:a1 - a0, off:512],
                                vt_v[:, i, a0:a1],
                                pt[:, sl * 512 + off:(sl + 1) * 512],
                                start=(i == 0), stop=(i == nm - 1))
                        if dbg and h == 0 and j == 0 and pr == 0:
                            nc.sync.dma_start(dpt[:], pt[:])
                    if dbg and h == 0 and j == 0:
                        dstg = rbpool.tile([_P, 512], fp32, tag="dstg")
                        nc.vector.tensor_copy(dstg[:], op[:])
                        nc.sync.dma_start(dop[:], dstg[:])
                    srt = rbpool.tile([1, 512], fp32, tag="srt")
                    nc.vector.tensor_copy(srt[:], op[srow:srow + 1, :])
                    rr = rbpool.tile([1, 512], fp32, tag="rr")
                    nc.vector.reciprocal_approx_fast(rr[:], srt[:])
                    rb = rbpool.tile([_P, 512], fp32, tag="rb")
                    nc.gpsimd.partition_broadcast(rb[:], rr[:])
                    nc.vector.tensor_tensor(
                        ot_v[orow0:orow0 + 64, h // 2, j * 512:(j + 1) * 512],
                        op[orow0:orow0 + 64, :], rb[orow0:orow0 + 64, :], OP.mult)

            if dbg:
                nc.sync.dma_start(dot[:], ot_all[:])

            # output projection (partial; host adds bo and reduces over cores)
            for i in range(_NT):
                ostg = ospool.tile([_P, _C], fp32)
                for e in range(2):
                    ps = mmps.tile([_P, 512], fp32, tag="ps")
                    for kk in range(2):
                        nc.tensor.matmul(
                            ps[:],
                            ot_v[:, kk, i * _P:(i + 1) * _P],
                            wo_sb[:, kk * _C + e * 512:kk * _C + (e + 1) * 512],
                            start=(kk == 0), stop=(kk == 1))
                    nc.vector.tensor_copy(ostg[:, e * 512:(e + 1) * 512], ps[:])
                nc.sync.dma_start(out_d[i * _P:(i + 1) * _P, :], ostg[:])

    nc.compile()
    return nc


def _get_program():
    if "nc" not in _CACHE:
        _CACHE["nc"] = _build_program()
    return _CACHE["nc"]


def _prep_inputs(x, Wq, bq, Wk, bk, Wv, bv, Wo, bo, gamma, beta, rel_bias):
    f8 = np.float64
    gamma8, beta8 = gamma.astype(f8), beta.astype(f8)
    M = _interp_matrix(_N, _S)
    Mz = M.copy()
    Mz[_N - 1, :] = 0.0
    mz_in = np.ascontiguousarray(Mz.T.astype(np.float32))

    in_maps = []
    for core in range(8):
        b, g = core // 4, core % 4
        hs = slice(g * _OC, (g + 1) * _OC)
        Wq_s, Wk_s, Wv_s = (W[hs, :].astype(f8) for W in (Wq, Wk, Wv))
        wq_in = np.ascontiguousarray((Wq_s * gamma8[None, :]).T * _SCALE)
        wk_in = np.ascontiguousarray((Wk_s * gamma8[None, :]).T)
        wv_in = np.ascontiguousarray((Wv_s * gamma8[None, :]).T)
        bq_eff = (bq[hs].astype(f8) + Wq_s @ beta8) * _SCALE
        bk_eff = bk[hs].astype(f8) + Wk_s @ beta8
        bv_eff = bv[hs].astype(f8) + Wv_s @ beta8
        bq_in = np.zeros((_P, _HPC), f8)
        bk_in = np.zeros((_P, _HPC), f8)
        for h in range(_HPC):
            r0 = (h % 2) * 64
            bq_in[r0:r0 + 64, h] = bq_eff[h * 64:(h + 1) * 64]
            bk_in[r0:r0 + 64, h] = bk_eff[h * 64:(h + 1) * 64]
        bvb_in = np.broadcast_to(bv_eff[None, :], (_P, _OC))
        wo_in = np.ascontiguousarray(Wo[:, hs].T.astype(f8))
        rt_in = np.zeros((_S, _HPC * _N), f8)
        for h in range(_HPC):
            rt_in[:, h * _N:(h + 1) * _N] = \
                rel_bias[g * _HPC + h].astype(f8) @ Mz.T.astype(f8)
        f = np.float32
        in_maps.append({
            "x": np.ascontiguousarray(x[b]).astype(f),
            "wq": wq_in.astype(f), "wk": wk_in.astype(f), "wv": wv_in.astype(f),
            "bq": bq_in.astype(f), "bk": bk_in.astype(f),
            "bvb": np.ascontiguousarray(bvb_in).astype(f),
            "wo": wo_in.astype(f), "mz": mz_in,
            "rt": rt_in.astype(f),
        })
    return in_maps


def _ensure_ntff_hook():
    """Register the axon NTFF profiling hook if the container lacks
    antenv.axon_hooks (degraded boot). Only used when tracing."""
    import sys
    import types
    try:
        from antenv.axon_hooks import get_axon_ntff_profile_hook  # noqa: F401
        return
    except ImportError:
        pass
    try:
        import antenv
        mod = types.ModuleType("antenv.axon_hooks")
        _reg = {}
        mod.set_axon_ntff_profile_hook = lambda h: _reg.__setitem__("h", h)
        mod.get_axon_ntff_profile_hook = lambda: _reg.get("h")
        sys.modules["antenv.axon_hooks"] = mod
        antenv.axon_hooks = mod
        from trn_agent_boot.trn_boot import _ntff_profile_via_ctypes
        hook = _ntff_profile_via_ctypes("/opt/axon/libaxon_pjrt.so")
        if hook is not None:
            mod.set_axon_ntff_profile_hook(hook)
    except Exception as e:  # profiling is best-effort
        print(f"ntff hook install failed: {e}")


def kernel(x, Wq, bq, Wk, bk, Wv, bv, Wo, bo, gamma, beta, rel_bias):
    global LAST_EXEC_NS
    args = [np.asarray(a, dtype=np.float32) for a in
            (x, Wq, bq, Wk, bk, Wv, bv, Wo, bo, gamma, beta, rel_bias)]
    x, Wq, bq, Wk, bk, Wv, bv, Wo, bo, gamma, beta, rel_bias = args

    from concourse.bass_utils import run_bass_kernel_spmd
    nc = _get_program()
    in_maps = _prep_inputs(x, Wq, bq, Wk, bk, Wv, bv, Wo, bo, gamma, beta,
                           rel_bias)
    trace = bool(int(os.environ.get("KBENCH_TRACE", "0")))
    if trace:
        _ensure_ntff_hook()
    tmpdir = os.environ.get("KBENCH_TRACE_DIR") or None
    res = run_bass_kernel_spmd(nc, in_maps, list(range(8)), trace=trace,
                               tmpdir=tmpdir)
    LAST_EXEC_NS = res.exec_time_ns

    out = np.empty((_B, _N, _C), np.float32)
    for b in range(_B):
        acc = np.zeros((_N, _C), np.float64)
        for g in range(4):
            acc += res.results[b * 4 + g]["out"].astype(np.float64)
        out[b] = (acc + bo.astype(np.float64)[None, :]).astype(np.float32)
    return out
